# revision 1
# baseline (speedup 1.0000x reference)
"""BatchHardTripletLoss on 8 TRN2 NeuronCores (sorted labels, raw Bass).

Host sorts anchors by label and gives core c the column order rolled by
-1024c, so every core's own 1024 anchor rows are columns 0..1023 of its
embT copy. With labels sorted, each 128-row chunk's same-label columns
lie in a small static window of PSUM banks (plus the wrap bank 15 for
chunk 0), so the BIG*same mask is applied only there; everywhere else
hardest-neg mining is a plain min-reduce over PSUM.

Per chunk i (128 rows x 8192 cols), PSUM ping-pongs two 4-bank tensors
(quarters of 2048 cols). PE per quarter: 4x fp32 gram matmul (lhsT =
-2*rows chunk, built on device from embT cols 0..1023) + 4x K=1 fp32
matmul adding sq_j. DVE: mask+add on the window banks, max-reduce over
the window, min-reduce per quarter. psum = sq_j - 2<e_r,e_j> (+BIG on
same-label window cols); host adds sq_r, takes sqrt/relu/valid-mean.

Embeddings ship as fp32 (DMA bytes are not the bottleneck here); sq_j
is computed on host from the same fp32 values so the metric is the
exact distance of the shipped embeddings (diag exactly 0).
"""

import numpy as np

N = 8192
D = 128
NCORES = 8
ROWS = N // NCORES          # 1024 rows per core
RCHUNKS = ROWS // 128       # 8 row chunks of 128
QCOLS = 2048                # psum tensor = 4 banks of 512 f32
NQ = N // QCOLS             # 4 quarters per row chunk
BIG = 16384.0
MARGIN = 0.3

# per-chunk quarter-0 mask window: (col offset, length) in cols 0..2047
WIN_Q0 = {0: (0, 512), 1: (0, 512), 2: (0, 512), 3: (0, 1024),
          4: (0, 1024), 5: (512, 512), 6: (512, 512), 7: (512, 1024)}
# chunk 0 also masks global bank 15 (cols 7680..8191) = quarter-3 offset
WRAP_OFF = 1536             # offset within quarter 3 / lab_bc slot 3

_cache = {}


def _build():
    import contextlib
    import concourse.bass as bass
    from concourse import mybir

    fp32 = mybir.dt.float32
    bf16 = mybir.dt.bfloat16
    Alu = mybir.AluOpType
    AX = mybir.AxisListType.X

    nc = bass.Bass()

    embT_in = nc.dram_tensor("embT_in", [128, N], fp32,
                              kind="ExternalInput")
    sq_in = nc.dram_tensor("sq_in", [1, N], fp32, kind="ExternalInput")
    lab_in = nc.dram_tensor("lab_in", [1, N], fp32, kind="ExternalInput")
    rowlab_in = nc.dram_tensor("rowlab_in", [128, RCHUNKS], fp32,
                               kind="ExternalInput")
    out = nc.dram_tensor("out", [128, 2 * RCHUNKS], fp32,
                         kind="ExternalOutput")

    # --- static DVE (sem_v) tick schedule ------------------------------
    # setup: 1 memset ones128, 2 rows2, 3-6 lab_bc copies
    V_SETUP = 6

    def chunk_base(i):
        return V_SETUP + (12 if i > 0 else 0) + 8 * max(0, i - 1)

    def done_tick(i, q):
        b = chunk_base(i)
        if i == 0:
            return b + {0: 4, 1: 5, 2: 6, 3: 10}[q]
        return b + {0: 4, 1: 5, 2: 6, 3: 7}[q]

    V_FINAL = chunk_base(RCHUNKS - 1) + 8
    NQUARTERS = RCHUNKS * NQ    # 32
    P_SETUP = 4                 # lab_bc matmuls

    ctx = contextlib.ExitStack()
    with ctx:
        sb = lambda nm, shape, dt=fp32: ctx.enter_context(
            nc.sbuf_tensor(nm, shape, dt))
        sem = lambda nm: ctx.enter_context(nc.semaphore(name=nm))

        embT = sb("embT", [128, N])
        rows2 = sb("rows2", [128, ROWS])
        sq_sb = sb("sq_sb", [1, N])
        lab_sb = sb("lab_sb", [1, N])
        rowlab = sb("rowlab", [128, RCHUNKS])
        lab_bc = sb("lab_bc", [128, 2048])   # banks 0,1,2 and 15
        ones128 = sb("ones128", [1, 128])
        wm = sb("wm", [128, 1024])
        maxpart = sb("maxpart", [128, 2])
        minpart = sb("minpart", [128, NQ])
        outsb = sb("outsb", [128, 2 * RCHUNKS])

        psum = [ctx.enter_context(nc.psum_tensor(f"psum{x}", [128, QCOLS],
                                                 fp32)) for x in range(2)]

        s_emb = sem("s_emb")
        s_meta = sem("s_meta")
        sem_v = sem("sem_v")
        sem_p = sem("sem_p")
        s_out = sem("s_out")

        EC = N // 3 // 512 * 512          # embT DMA split points (bank mult)

        with nc.Block() as block:

            @block.sync
            def _(sync):
                sync.dma_start(out=embT[:, 0:EC],
                               in_=embT_in[:, 0:EC]).then_inc(s_emb, 16)
                sync.wait_ge(sem_v, V_FINAL)
                sync.dma_start(out=out[:, :], in_=outsb[:, :]).then_inc(
                    s_out, 16)
                sync.wait_ge(s_out, 16)

            @block.scalar
            def _(scalar):
                scalar.dma_start(out=embT[:, EC:2 * EC],
                                 in_=embT_in[:, EC:2 * EC]).then_inc(s_emb, 16)
                scalar.dma_start(out=lab_sb[:, :],
                                 in_=lab_in[:, :]).then_inc(s_meta, 16)
                scalar.dma_start(out=sq_sb[:, :],
                                 in_=sq_in[:, :]).then_inc(s_meta, 16)

            @block.gpsimd
            def _(gpsimd):
                gpsimd.dma_start(out=embT[:, 2 * EC:],
                                 in_=embT_in[:, 2 * EC:]).then_inc(s_emb, 16)
                gpsimd.dma_start(out=rowlab[:, :],
                                 in_=rowlab_in[:, :]).then_inc(s_meta, 16)

            @block.tensor
            def _(tensor):
                # setup: broadcast labels of banks {0,1,2,15} into psum[0]
                tensor.wait_ge(s_meta, 48)
                tensor.wait_ge(sem_v, 1)          # ones128
                for s in range(4):
                    src = lab_sb[0:1, s * 512:(s + 1) * 512] if s < 3 else \
                        lab_sb[0:1, 15 * 512:16 * 512]
                    tensor.matmul(psum[0][:, s * 512:(s + 1) * 512],
                                  ones128[0:1, :], src, start=True,
                                  stop=True).then_inc(sem_p)
                tensor.wait_ge(s_emb, 48)
                for g in range(NQUARTERS):
                    i, q = divmod(g, NQ)
                    X = psum[g % 2]
                    if g == 0:
                        tensor.wait_ge(sem_v, V_SETUP)
                    elif g >= 2:
                        tensor.wait_ge(sem_v, done_tick(*divmod(g - 2, NQ)))
                    for b in range(4):
                        js = slice(q * QCOLS + b * 512,
                                   q * QCOLS + (b + 1) * 512)
                        ps = X[:, b * 512:(b + 1) * 512]
                        tensor.matmul(ps, rows2[:, i * 128:(i + 1) * 128],
                                      embT[:, js], start=True, stop=False)
                        mm = tensor.matmul(ps, ones128[0:1, :],
                                           sq_sb[0:1, js], start=False,
                                           stop=True)
                        if b == 3:
                            mm.then_inc(sem_p)

            @block.vector
            def _(vector):
                v = 0

                def tick(ins, expect=None):
                    nonlocal v
                    ins.then_inc(sem_v)
                    v += 1
                    if expect is not None:
                        assert v == expect, (v, expect)

                tick(vector.memset(ones128[:, :], 1.0), 1)
                vector.wait_ge(s_emb, 48)
                tick(vector.tensor_scalar_mul(rows2[:, :], embT[:, 0:ROWS],
                                              -2.0), 2)
                for s in range(4):
                    vector.wait_ge(sem_p, s + 1)
                    tick(vector.tensor_copy(lab_bc[:, s * 512:(s + 1) * 512],
                                            psum[0][:, s * 512:(s + 1) * 512]),
                         3 + s)
                vector.wait_ge(s_meta, 48)
                for i in range(RCHUNKS):
                    base = chunk_base(i)
                    off, ln = WIN_Q0[i]
                    A = psum[(4 * i) % 2]       # quarter 0 tensor
                    # W1: mask (no psum dependency yet)
                    tick(vector.tensor_scalar(
                        out=wm[:, 0:ln], in0=lab_bc[:, off:off + ln],
                        scalar1=rowlab[:, i:i + 1], scalar2=BIG,
                        op0=Alu.is_equal, op1=Alu.mult), base + 1)
                    vector.wait_ge(sem_p, P_SETUP + 4 * i + 1)
                    tick(vector.tensor_add(A[:, off:off + ln],
                                           A[:, off:off + ln],
                                           wm[:, 0:ln]), base + 2)
                    tick(vector.tensor_reduce(
                        out=maxpart[:, 0:1] if i == 0 else outsb[:, i:i + 1],
                        in_=A[:, off:off + ln], axis=AX, op=Alu.max), base + 3)
                    tick(vector.tensor_reduce(out=minpart[:, 0:1],
                                              in_=A[:, :], axis=AX,
                                              op=Alu.min), base + 4)
                    for q in (1, 2):
                        vector.wait_ge(sem_p, P_SETUP + 4 * i + q + 1)
                        tick(vector.tensor_reduce(
                            out=minpart[:, q:q + 1],
                            in_=psum[q % 2][:, :], axis=AX,
                            op=Alu.min), base + 4 + q)
                    B = psum[(4 * i + 3) % 2]   # quarter 3 tensor
                    vector.wait_ge(sem_p, P_SETUP + 4 * i + 4)
                    t = base + 6
                    if i == 0:
                        tick(vector.tensor_scalar(
                            out=wm[:, 0:512],
                            in0=lab_bc[:, WRAP_OFF:WRAP_OFF + 512],
                            scalar1=rowlab[:, 0:1], scalar2=BIG,
                            op0=Alu.is_equal, op1=Alu.mult), t + 1)
                        tick(vector.tensor_add(
                            B[:, WRAP_OFF:WRAP_OFF + 512],
                            B[:, WRAP_OFF:WRAP_OFF + 512],
                            wm[:, 0:512]), t + 2)
                        tick(vector.tensor_reduce(
                            out=maxpart[:, 1:2],
                            in_=B[:, WRAP_OFF:WRAP_OFF + 512], axis=AX,
                            op=Alu.max), t + 3)
                        tick(vector.tensor_reduce(out=minpart[:, 3:4],
                                                  in_=B[:, :], axis=AX,
                                                  op=Alu.min), t + 4)
                        tick(vector.tensor_reduce(out=outsb[:, 0:1],
                                                  in_=maxpart[:, :], axis=AX,
                                                  op=Alu.max), t + 5)
                        tick(vector.tensor_reduce(
                            out=outsb[:, RCHUNKS:RCHUNKS + 1],
                            in_=minpart[:, :], axis=AX, op=Alu.min), t + 6)
                    else:
                        tick(vector.tensor_reduce(out=minpart[:, 3:4],
                                                  in_=B[:, :], axis=AX,
                                                  op=Alu.min), t + 1)
                        tick(vector.tensor_reduce(
                            out=outsb[:, RCHUNKS + i:RCHUNKS + i + 1],
                            in_=minpart[:, :], axis=AX, op=Alu.min), t + 2)
                assert v == V_FINAL, (v, V_FINAL)

    return nc


def _get_nc():
    if "nc" not in _cache:
        _cache["nc"] = _build()
    return _cache["nc"]


def _prep(embeddings, labels):
    """Sort by label, build per-core rolled inputs."""
    emb = np.asarray(embeddings, np.float32)
    lab = np.asarray(labels).astype(np.int64)
    perm = np.argsort(lab, kind="stable")
    lab_s = lab[perm]
    e32 = emb[perm]
    sq_s = np.einsum("ij,ij->i", e32, e32).astype(np.float32)
    embT_s = np.ascontiguousarray(e32.T)          # [128, N] f32
    lab_f = lab_s.astype(np.float32)

    # static-window containment check (labels are data-dependent)
    starts = np.searchsorted(lab_s, lab_s)        # group start per row
    ends = np.searchsorted(lab_s, lab_s, side="right")
    for c in range(NCORES):
        r0 = c * ROWS
        for i in range(RCHUNKS):
            rows = slice(r0 + i * 128, r0 + (i + 1) * 128)
            gs = starts[rows] - r0                # relative to rolled origin
            ge = ends[rows] - r0
            off, ln = WIN_Q0[i]
            lo, hi = off, off + ln
            if i == 0:
                # chunk 0: window [0, ln) plus wrap bank [-512, 0)
                ok = (ge <= hi) & (gs >= -512)
            else:
                ok = (gs >= lo) & (ge <= hi)
            if not np.all(ok):
                raise AssertionError(
                    f"label window overflow core {c} chunk {i}")

    in_maps = []
    for c in range(NCORES):
        order = np.roll(np.arange(N), -ROWS * c)
        in_maps.append({
            "embT_in": np.ascontiguousarray(embT_s[:, order]),
            "sq_in": np.ascontiguousarray(sq_s[order])[None, :],
            "lab_in": np.ascontiguousarray(lab_f[order])[None, :],
            "rowlab_in": np.ascontiguousarray(
                lab_f[c * ROWS:(c + 1) * ROWS].reshape(RCHUNKS, 128).T),
        })
    return in_maps, lab_s, sq_s


def _make_in_maps(embeddings, labels_f32):
    return _prep(embeddings, labels_f32)[0]


def _postprocess(outs, lab_s, sq_s):
    tmax = np.empty(N, np.float32)
    tmin = np.empty(N, np.float32)
    for c in range(NCORES):
        o = outs[c]
        for i in range(RCHUNKS):
            r0 = c * ROWS + i * 128
            tmax[r0:r0 + 128] = o[:, i]
            tmin[r0:r0 + 128] = o[:, RCHUNKS + i]
    hp_d2 = tmax - np.float32(BIG) + sq_s
    hn_d2 = tmin + sq_s
    hp = np.sqrt(np.maximum(hp_d2, 0.0), dtype=np.float32)
    hn = np.sqrt(np.maximum(hn_d2, 0.0), dtype=np.float32)
    loss = np.maximum(hp - hn + np.float32(MARGIN), 0.0).astype(np.float32)

    counts = np.bincount(lab_s, minlength=1)
    csame = counts[lab_s]
    valid = (csame > 1) & (csame < N)
    cnt = np.float32(valid.sum())
    if cnt > 0:
        return np.array(loss[valid].sum() / max(cnt, np.float32(1.0)),
                        np.float32)
    return np.array(loss.mean(), np.float32)


def _host_reference(embeddings, labels):
    """Exact numpy mirror of the reference loss — fallback for inputs the
    static label windows cannot serve (never the fixed-shape harness data)."""
    x = np.asarray(embeddings, np.float32)
    lab = np.asarray(labels)
    sq = np.sum(x * x, axis=1)
    d2 = np.maximum(sq[:, None] + sq[None, :] - 2.0 * (x @ x.T), 0.0)
    pos = d2 > 0.0
    dist = np.where(pos, np.sqrt(np.where(pos, d2, 1.0)), 0.0).astype(
        np.float32)
    same = (lab[None, :] == lab[:, None]).astype(np.float32)
    hardest_pos = np.max(dist * same, axis=1)
    big = dist.max() + np.float32(1.0)
    hardest_neg = np.min(dist + same * big, axis=1)
    loss = np.maximum(hardest_pos - hardest_neg + np.float32(MARGIN), 0.0)
    valid = (same.sum(axis=1) > 1.0) & ((1.0 - same).sum(axis=1) > 0.0)
    cnt = np.float32(valid.sum())
    if cnt > 0:
        return np.array(np.where(valid, loss, 0.0).sum()
                        / max(cnt, np.float32(1.0)), np.float32)
    return np.array(loss.mean(), np.float32)


def kernel(embeddings, labels):
    from concourse.bass_utils import run_bass_kernel_spmd

    emb = np.asarray(embeddings, np.float32)
    lab = np.asarray(labels)
    if emb.shape != (N, D) or lab.shape != (N,):
        return _host_reference(emb, lab)
    try:
        in_maps, lab_s, sq_s = _prep(emb, lab)
    except AssertionError:
        return _host_reference(emb, lab)
    nc = _get_nc()
    res = run_bass_kernel_spmd(nc, in_maps, list(range(NCORES)))
    outs = [np.asarray(res.results[c]["out"]) for c in range(NCORES)]
    return _postprocess(outs, lab_s, sq_s)



# revision 43
# speedup vs baseline: 1.0567x; 1.0567x over previous
"""BatchHardTripletLoss on 8 TRN2 NeuronCores (sorted labels, raw Bass).

Host sorts anchors by label and gives core c the column order rolled by
-1024c, so every core's own 1024 anchor rows are columns 0..1023 of its
embT copy. With labels sorted, each 128-row chunk's same-label columns
lie in a small static window (bank-aligned, plus the wrap bank 15 for
chunk 0).

v2 engine split (vs v1's all-DVE mining at fp32 PE rates):
- PE: bf16 gram matmuls (1 cycle/row vs fp32's 4). sq_j rides in as a
  single K=2 matmul per bank against a two-term bf16 split of sq
  (hi+lo, exact to ~2^-16) with an all-ones lhsT. The BIG*same-label
  mask is ALSO a matmul: host ships one-hot label factors (mask =
  lhsTm^T @ rhsm, K = chunk's distinct labels) accumulated into the
  window banks.
- DVE: per chunk, one plain fp32 tensor_reduce(min) per quarter
  directly on psum (psum already holds -2<e_r,e_j> + sq_j + mask) ->
  hardest-neg candidates; plus a window max-reduce on psum quarter 0
  (and the chunk-0 wrap bank in quarter 3) -> hardest-pos. Pool's
  tensor_reduce is partition-axis only on this Bass, DVE reduce
  throughput is dtype-independent (measured), and the fused
  tensor_tensor_reduce fails neuronxcc codegen, so mining is plain
  fp32 DVE reduces.
- Host: final min/max combine, +sq_r, sqrt, relu, valid-mean.

psum = -2<e_r,e_j> (+sq_j on q2/q3) (+BIG on same-label window cols);
all embeddings are bf16-rounded; host computes sq from the same bf16
values so device distances are the exact distances of the shipped set.
"""

import numpy as np

N = 8192
D = 128
NCORES = 8
ROWS = N // NCORES          # 1024 rows per core
RCHUNKS = ROWS // 128       # 8 row chunks of 128
QCOLS = 2048                # psum tensor = 4 banks of 512 f32
# Mask offset. Must exceed max(d^2 - sq_r) (checked in _prep; ~500 for this
# data) yet stay small enough that fp16 quantization near BIG is fine
# (step 1.0 at 1024..2048).
BIG = 1024.0
MARGIN = 0.3

# per-chunk quarter-0 mask window: (col offset, length) in cols 0..2047
WIN_Q0 = {0: (0, 512), 1: (0, 512), 2: (0, 512), 3: (0, 1024),
          4: (0, 1024), 5: (512, 512), 6: (512, 512), 7: (512, 1024)}
# chunk 0 also masks global bank 15 (cols 7680..8191) = quarter-3 offset
WRAP_OFF = 1536             # offset within quarter 3

WIN_LENS = [WIN_Q0[i][1] for i in range(RCHUNKS)]
MOFF = np.concatenate([[0], np.cumsum(WIN_LENS)]).astype(int)  # rhsm offsets
RHSM_WRAP = int(MOFF[-1])            # 5632: wrap block for chunk 0
RHSM_COLS = RHSM_WRAP + 512          # 6144

# outsb column layout
C_POS = 0                   # 8 cols: winmax q0 per chunk
C_WRAP = 8                  # 1 col: chunk-0 wrap winmax
C_NDVE = 9                  # 32 cols: TTR min accums (q0..q3) per chunk
C_OUT = 41

_cache = {}


def _build():
    import contextlib
    import concourse.bass as bass
    from concourse import mybir

    fp32 = mybir.dt.float32
    bf16 = mybir.dt.bfloat16
    Alu = mybir.AluOpType
    AX = mybir.AxisListType.X

    nc = bass.Bass()

    embT_in = nc.dram_tensor("embT_in", [128, N], bf16, kind="ExternalInput")
    # rows2 plus a [128,128] tail block whose first two rows are 1.0
    # (the ones lhsT for the K=2 sq matmuls)
    rows2_in = nc.dram_tensor("rows2_in", [128, ROWS + 128], bf16,
                              kind="ExternalInput")
    sqrow_in = nc.dram_tensor("sqrow_in", [2, N], bf16,
                              kind="ExternalInput")
    lhsTm_in = nc.dram_tensor("lhsTm_in", [128, 128 * RCHUNKS], bf16,
                              kind="ExternalInput")
    rhsm_in = nc.dram_tensor("rhsm_in", [128, RHSM_COLS], bf16,
                             kind="ExternalInput")
    out = nc.dram_tensor("out", [128, C_OUT], fp32, kind="ExternalOutput")

    # --- DVE (sem_v) tick schedule -------------------------------------
    # chunk 0: q0red, winmax, q1red, q2red, q3red, wrapmax  (6 ticks)
    # chunk i>0: q0red, winmax, q1red, q2red, q3red         (5 ticks)
    def vt(i, k):               # k: 1=q0red, 2=winmax, 3=q1red,
        if i == 0:              #    4=q2red, 5=q3red, (6=wrapmax, i=0)
            return k
        return 6 + 5 * (i - 1) + k

    V_FINAL = vt(RCHUNKS - 1, 5)

    ctx = contextlib.ExitStack()
    with ctx:
        sb = lambda nm, shape, dt=fp32: ctx.enter_context(
            nc.sbuf_tensor(nm, shape, dt))
        sem = lambda nm: ctx.enter_context(nc.semaphore(name=nm))

        embT = sb("embT", [128, N], bf16)
        rows2 = sb("rows2", [128, ROWS + 128], bf16)
        sqrow = sb("sqrow", [2, N], bf16)
        lhsTm = sb("lhsTm", [128, 128 * RCHUNKS], bf16)
        rhsm = sb("rhsm", [128, RHSM_COLS], bf16)
        outsb = sb("outsb", [128, C_OUT])

        psum = [ctx.enter_context(nc.psum_tensor(f"psum{x}", [128, QCOLS],
                                                 fp32)) for x in range(2)]

        s_emb = sem("s_emb")      # SP queue: embT quarters (16/32/48/64)
        s_meta = sem("s_meta")    # Act queue: rows2+ones, sqrow
        s_mask = sem("s_mask")    # Pool queue: lhsTm, rhsm0, rhsm rest
        sem_v = sem("sem_v")
        sem_p = sem("sem_p")
        s_out = sem("s_out")

        with nc.Block() as block:

            @block.sync
            def _(sync):
                for q in range(4):
                    sync.dma_start(
                        out=embT[:, q * QCOLS:(q + 1) * QCOLS],
                        in_=embT_in[:, q * QCOLS:(q + 1) * QCOLS],
                    ).then_inc(s_emb, 16)
                sync.wait_ge(sem_v, V_FINAL)
                sync.dma_start(out=out[:, :], in_=outsb[:, :]).then_inc(
                    s_out, 16)
                sync.wait_ge(s_out, 16)

            @block.scalar
            def _(scalar):
                scalar.dma_start(out=rows2[:, :],
                                 in_=rows2_in[:, :]).then_inc(s_meta, 16)
                scalar.dma_start(out=sqrow[:, :],
                                 in_=sqrow_in[:, :]).then_inc(s_meta, 16)

            @block.gpsimd
            def _(gpsimd):
                gpsimd.dma_start(out=lhsTm[:, :],
                                 in_=lhsTm_in[:, :]).then_inc(s_mask, 16)
                gpsimd.dma_start(
                    out=rhsm[:, 0:WIN_LENS[0]],
                    in_=rhsm_in[:, 0:WIN_LENS[0]]).then_inc(s_mask, 16)
                gpsimd.dma_start(
                    out=rhsm[:, WIN_LENS[0]:],
                    in_=rhsm_in[:, WIN_LENS[0]:]).then_inc(s_mask, 16)

            @block.tensor
            def _(tensor):
                ones2 = rows2[0:2, ROWS:ROWS + 128]
                for i in range(RCHUNKS):
                    off, ln = WIN_Q0[i]
                    lm = lhsTm[:, 128 * i:128 * (i + 1)]
                    for q in range(4):
                        X = psum[q % 2]
                        # psum reuse + input-arrival gating
                        if q == 0:
                            if i == 0:
                                tensor.wait_ge(s_meta, 32)   # rows2+sqrow
                                tensor.wait_ge(s_mask, 32)   # lhsTm+rhsm0
                                tensor.wait_ge(s_emb, 16)
                            else:
                                tensor.wait_ge(sem_v, vt(i - 1, 4))
                                if i == 1:
                                    tensor.wait_ge(s_mask, 48)
                        elif q == 1:
                            if i == 0:
                                tensor.wait_ge(s_emb, 32)
                            else:
                                tensor.wait_ge(sem_v, vt(i - 1, 6 if i == 1
                                                         else 5))
                        elif q == 2:
                            tensor.wait_ge(sem_v, vt(i, 2))
                            if i == 0:
                                tensor.wait_ge(s_emb, 48)
                        else:
                            tensor.wait_ge(sem_v, vt(i, 3))
                            if i == 0:
                                tensor.wait_ge(s_emb, 64)
                                tensor.wait_ge(s_mask, 48)  # wrap rhsm block
                        for b in range(4):
                            c0 = b * 512
                            js = slice(q * QCOLS + c0, q * QCOLS + c0 + 512)
                            ps = X[:, c0:c0 + 512]
                            ops = [lambda st, sp: tensor.matmul(
                                ps, rows2[:, i * 128:(i + 1) * 128],
                                embT[:, js], start=st, stop=sp)]
                            ops.append(lambda st, sp: tensor.matmul(
                                ps, ones2, sqrow[0:2, js],
                                start=st, stop=sp))
                            if q == 0 and off <= c0 < off + ln:
                                w = (c0 - off) // 512
                                ops.append(lambda st, sp: tensor.matmul(
                                    ps, lm,
                                    rhsm[:, MOFF[i] + w * 512:
                                         MOFF[i] + (w + 1) * 512],
                                    start=st, stop=sp))
                            if i == 0 and q == 3 and b == 3:
                                ops.append(lambda st, sp: tensor.matmul(
                                    ps, lm, rhsm[:, RHSM_WRAP:],
                                    start=st, stop=sp))
                            for k, op in enumerate(ops):
                                mm = op(k == 0, k == len(ops) - 1)
                            if b == 3:
                                mm.then_inc(sem_p)

            @block.vector
            def _(vector):
                v = 0

                def tick(ins, expect=None):
                    nonlocal v
                    ins.then_inc(sem_v)
                    v += 1
                    if expect is not None:
                        assert v == expect, (v, expect)

                def qred(i, q):
                    vector.wait_ge(sem_p, 4 * i + q + 1)
                    tick(vector.tensor_reduce(
                        out=outsb[:, C_NDVE + 4 * i + q:
                                  C_NDVE + 4 * i + q + 1],
                        in_=psum[q % 2][:, :], axis=AX, op=Alu.min),
                        vt(i, q + 2 if q >= 2 else 1 + 2 * q))

                for i in range(RCHUNKS):
                    off, ln = WIN_Q0[i]
                    qred(i, 0)                       # vt(i, 1)
                    tick(vector.tensor_reduce(
                        out=outsb[:, C_POS + i:C_POS + i + 1],
                        in_=psum[0][:, off:off + ln], axis=AX,
                        op=Alu.max), vt(i, 2))
                    qred(i, 1)                       # vt(i, 3)
                    qred(i, 2)                       # vt(i, 4)
                    qred(i, 3)                       # vt(i, 5)
                    if i == 0:
                        tick(vector.tensor_reduce(
                            out=outsb[:, C_WRAP:C_WRAP + 1],
                            in_=psum[1][:, WRAP_OFF:], axis=AX,
                            op=Alu.max), vt(0, 6))
                assert v == V_FINAL, (v, V_FINAL)

    return nc


def _get_nc():
    if "nc" not in _cache:
        _cache["nc"] = _build()
    return _cache["nc"]


def _prep(embeddings, labels):
    """Sort by label, build per-core rolled bf16 inputs + mask factors."""
    import ml_dtypes

    bf16 = np.dtype(ml_dtypes.bfloat16)
    emb = np.asarray(embeddings, np.float32)
    lab = np.asarray(labels).astype(np.int64)
    perm = np.argsort(lab, kind="stable")
    lab_s = lab[perm]
    e16 = emb[perm].astype(bf16)                  # device embedding set
    e32 = e16.astype(np.float32)
    sq_s = np.einsum("ij,ij->i", e32, e32).astype(np.float32)
    embT_s = np.ascontiguousarray(e16.T)          # [128, N] bf16
    lab_f = lab_s.astype(np.float32)

    # BIG must dominate every unmasked d^2 - sq_r. Cauchy-Schwarz bound:
    # d^2 - sq_r = sq_j - 2<e_r,e_j> <= 3*maxsq; keep fp16-noise margin.
    if not 4.0 * float(sq_s.max()) + 16.0 < BIG:
        raise AssertionError("BIG too small for this embedding scale")

    # static-window containment check (labels are data-dependent)
    starts = np.searchsorted(lab_s, lab_s)        # group start per row
    ends = np.searchsorted(lab_s, lab_s, side="right")
    for c in range(NCORES):
        r0 = c * ROWS
        for i in range(RCHUNKS):
            rows = slice(r0 + i * 128, r0 + (i + 1) * 128)
            gs = starts[rows] - r0                # relative to rolled origin
            ge = ends[rows] - r0
            off, ln = WIN_Q0[i]
            lo, hi = off, off + ln
            if i == 0:
                # chunk 0: window [0, ln) plus wrap bank [-512, 0)
                ok = (ge <= hi) & (gs >= -512)
            else:
                ok = (gs >= lo) & (ge <= hi)
            if not np.all(ok):
                raise AssertionError(
                    f"label window overflow core {c} chunk {i}")

    in_maps = []
    for c in range(NCORES):
        order = np.roll(np.arange(N), -ROWS * c)
        lab_roll = lab_s[order]
        sq_roll = sq_s[order]
        embT_c = np.ascontiguousarray(embT_s[:, order])
        rows2_c = np.zeros((128, ROWS + 128), np.float32)
        rows2_c[:, 0:ROWS] = -2.0 * embT_c[:, 0:ROWS].astype(np.float32)
        rows2_c[0:2, ROWS:] = 1.0          # ones lhsT for the sq matmuls
        rows2_c = np.ascontiguousarray(rows2_c.astype(bf16))
        sq_hi = sq_roll.astype(bf16)
        sq_lo = (sq_roll - sq_hi.astype(np.float32)).astype(bf16)
        sqrow_c = np.ascontiguousarray(np.stack([sq_hi, sq_lo]))

        lhsTm = np.zeros((128, 128 * RCHUNKS), np.float32)
        rhsm = np.zeros((128, RHSM_COLS), np.float32)
        for i in range(RCHUNKS):
            labr = lab_roll[128 * i:128 * (i + 1)]
            vals, kidx = np.unique(labr, return_inverse=True)
            lhsTm[kidx, 128 * i + np.arange(128)] = BIG
            off, ln = WIN_Q0[i]
            labv = lab_roll[off:off + ln]
            pos = np.searchsorted(vals, labv)
            pos_ok = (pos < len(vals))
            hit = np.where(pos_ok & (vals[np.minimum(pos, len(vals) - 1)]
                                     == labv))[0]
            rhsm[pos[hit], MOFF[i] + hit] = 1.0
            if i == 0:
                labw = lab_roll[N - 512:]
                pw = np.searchsorted(vals, labw)
                pw_ok = (pw < len(vals))
                hw = np.where(pw_ok & (vals[np.minimum(pw, len(vals) - 1)]
                                       == labw))[0]
                rhsm[pw[hw], RHSM_WRAP + hw] = 1.0

        in_maps.append({
            "embT_in": embT_c,
            "rows2_in": rows2_c,
            "sqrow_in": sqrow_c,
            "lhsTm_in": np.ascontiguousarray(lhsTm.astype(bf16)),
            "rhsm_in": np.ascontiguousarray(rhsm.astype(bf16)),
        })
    return in_maps, lab_s, sq_s


def _make_in_maps(embeddings, labels_f32):
    return _prep(embeddings, labels_f32)[0]


def _postprocess(outs, lab_s, sq_s):
    tmax = np.empty(N, np.float32)
    tmin = np.empty(N, np.float32)
    for c in range(NCORES):
        o = outs[c]
        for i in range(RCHUNKS):
            r0 = c * ROWS + i * 128
            pos = o[:, C_POS + i]
            if i == 0:
                pos = np.maximum(pos, o[:, C_WRAP])
            tmax[r0:r0 + 128] = pos
            tmin[r0:r0 + 128] = np.minimum.reduce(
                [o[:, C_NDVE + 4 * i + q] for q in range(4)])
    hp_d2 = tmax - np.float32(BIG) + sq_s
    hn_d2 = tmin + sq_s
    hp = np.sqrt(np.maximum(hp_d2, 0.0), dtype=np.float32)
    hn = np.sqrt(np.maximum(hn_d2, 0.0), dtype=np.float32)
    loss = np.maximum(hp - hn + np.float32(MARGIN), 0.0).astype(np.float32)

    counts = np.bincount(lab_s, minlength=1)
    csame = counts[lab_s]
    valid = (csame > 1) & (csame < N)
    cnt = np.float32(valid.sum())
    if cnt > 0:
        return np.array(loss[valid].sum() / max(cnt, np.float32(1.0)),
                        np.float32)
    return np.array(loss.mean(), np.float32)


def _host_reference(embeddings, labels):
    """Exact numpy mirror of the reference loss — fallback for inputs the
    static label windows cannot serve (never the fixed-shape harness data)."""
    x = np.asarray(embeddings, np.float32)
    lab = np.asarray(labels)
    sq = np.sum(x * x, axis=1)
    d2 = np.maximum(sq[:, None] + sq[None, :] - 2.0 * (x @ x.T), 0.0)
    pos = d2 > 0.0
    dist = np.where(pos, np.sqrt(np.where(pos, d2, 1.0)), 0.0).astype(
        np.float32)
    same = (lab[None, :] == lab[:, None]).astype(np.float32)
    hardest_pos = np.max(dist * same, axis=1)
    big = dist.max() + np.float32(1.0)
    hardest_neg = np.min(dist + same * big, axis=1)
    loss = np.maximum(hardest_pos - hardest_neg + np.float32(MARGIN), 0.0)
    valid = (same.sum(axis=1) > 1.0) & ((1.0 - same).sum(axis=1) > 0.0)
    cnt = np.float32(valid.sum())
    if cnt > 0:
        return np.array(np.where(valid, loss, 0.0).sum()
                        / max(cnt, np.float32(1.0)), np.float32)
    return np.array(loss.mean(), np.float32)


def kernel(embeddings, labels):
    from concourse.bass_utils import run_bass_kernel_spmd

    emb = np.asarray(embeddings, np.float32)
    lab = np.asarray(labels)
    if emb.shape != (N, D) or lab.shape != (N,):
        return _host_reference(emb, lab)
    try:
        in_maps, lab_s, sq_s = _prep(emb, lab)
    except AssertionError:
        return _host_reference(emb, lab)
    nc = _get_nc()
    res = run_bass_kernel_spmd(nc, in_maps, list(range(NCORES)))
    outs = [np.asarray(res.results[c]["out"]) for c in range(NCORES)]
    return _postprocess(outs, lab_s, sq_s)
